# revision 16
# baseline (speedup 1.0000x reference)
"""CKA vq_codebook kernel for 8 Trainium2 NeuronCores.

Math (verified against the reference in fp64):
  Gx[b]  = x[b] @ x[b]^T, Cxc = double-center(Gx) = (P x[b]) (P x[b])^T
  Gy[m]  = cent[m] @ cent[m]^T           (uncentered)
  hsic[b,m] = <Cxc[b], center(Gy[m])> = <Cxc[b], Gy[m]>   (centering is a
              self-adjoint projection and Cxc is already centered)
  vx[b] = ||Cxc[b]||_F
  vy[m] = ||center(Gy[m])||_F = sqrt(Q - (2/L)*sum_i R_i^2 + (S/L)^2)
          with Q = ||Gy||_F^2, R = row sums of Gy, S = total sum
  The second CKA pass is a pure gather: Cs = Cy[idx]  =>  hsic2 = hsic[:, idx],
  vs = vy[idx].

Sharding: codebook M=512 split 64 per core (expert-style); every core
computes all B=32 input grams (cheaper than an AllGather on this fabric —
the 2MB collective measured ~90us end-to-end).  Each core computes a 96x64
block of the Gram of the flattened 128x128 matrices
[Gy_0..Gy_63 | Cxc_0..Cxc_31] against the 64 Gy ones: rows 64..96 give hsic,
the leading diagonal gives ||Gy||^2.  Host does the O(B*M) normalization /
log / argmax tail and the gathered second pass.

Layout tricks: the PE contracts over partitions only, so the gram G = A A^T
needs A^T (h-major) tiles in SBUF — the host pre-transposes x and the
centroids into the exact SBUF images so the device never transposes anything.
The flattened-tile matrix ("big") is stored i-major (contraction chunk major)
so the 128 accumulating Gram matmuls read contiguous SBUF columns.
"""

import os

import numpy as np

# Recover automatically if a previous process left a NeuronCore wedged; takes
# effect at NRT init (which happens after import, on first device use).
os.environ.setdefault("NEURON_RT_RESET_CORES", "1")

B, L, H, M = 32, 128, 512, 512
N_CORES = 8
MLOC = M // N_CORES          # 64 centroids per core
G = H // 128                 # 4 contraction chunks of 128
NV = B + MLOC                # 96 vectors in the per-core big Gram
EPS = 1e-8

_BUILT = {}


def _build_bass():
    """Build the single-core Bass/Tile program (same program on all 8 cores)."""
    from concourse import bacc, mybir
    from concourse.tile import TileContext

    f32 = mybir.dt.float32
    nc = bacc.Bacc("TRN2", target_bir_lowering=False, debug=False,
                   num_devices=N_CORES)

    xt = nc.dram_tensor("xt", [B, 128, G * 128], f32, kind="ExternalInput")
    ct = nc.dram_tensor("ct", [MLOC, 128, G * 128], f32, kind="ExternalInput")
    bg_out = nc.dram_tensor("bg", [MLOC, 2 * NV], f32, kind="ExternalOutput")
    rt_out = nc.dram_tensor("rt", [128, MLOC + B], f32, kind="ExternalOutput")

    with TileContext(nc) as tc:
        with (
            tc.tile_pool(name="big", bufs=1) as bigp,
            tc.tile_pool(name="stage", bufs=8) as stage,
            tc.tile_pool(name="cxs", bufs=4) as cxs,
            tc.tile_pool(name="psg", bufs=4, space="PSUM") as psg,
            tc.tile_pool(name="psbg", bufs=1, space="PSUM") as psbg,
            tc.tile_pool(name="outp", bufs=1) as outp,
        ):
            # i-major: flattened-tile vector v's chunk i lives at free column
            # i*NV + v.  Vectors 0..64 = local Gy, 64..96 = global Cxc.
            big = bigp.tile([128, 128 * NV], f32)
            big3 = big[:].rearrange("p (i v) -> p i v", v=NV)
            rt = outp.tile([128, MLOC + B], f32)

            def gram(src_dram):
                st = stage.tile([128, G * 128], f32, tag="stage")
                nc.gpsimd.dma_start(st[:], src_dram)
                ps = psg.tile([128, 128], f32, tag="gram")
                for g in range(G):
                    sl = st[:, 128 * g:128 * (g + 1)]
                    nc.tensor.matmul(ps[:], sl, sl,
                                     start=(g == 0), stop=(g == G - 1))
                return ps

            # ---- Cxc grams (all B, replicated) + vx^2 stats ----
            for b in range(B):
                ps = gram(xt[b])
                v = MLOC + b
                if b % 2 == 0:
                    nc.scalar.activation(big3[:, :, v], ps[:],
                                         mybir.ActivationFunctionType.Copy)
                else:
                    nc.vector.tensor_copy(big3[:, :, v], ps[:])
                scr = cxs.tile([128, 128], f32, tag="scr")
                nc.scalar.activation(scr[:], ps[:],
                                     mybir.ActivationFunctionType.Square)
                nc.vector.tensor_reduce(
                    rt[:, MLOC + b:MLOC + b + 1], scr[:],
                    axis=mybir.AxisListType.X, op=mybir.AluOpType.add)

            # ---- local Gy grams + row sums ----
            for m in range(MLOC):
                ps = gram(ct[m])
                if m % 2 == 0:
                    nc.scalar.activation(big3[:, :, m], ps[:],
                                         mybir.ActivationFunctionType.Copy)
                else:
                    nc.vector.tensor_copy(big3[:, :, m], ps[:])
                nc.vector.tensor_reduce(
                    rt[:, m:m + 1], ps[:],
                    axis=mybir.AxisListType.X, op=mybir.AluOpType.add)

            # ---- big Gram: [64 Gy] x [all 96] over 128 contraction chunks.
            # The 64 Gy vectors are the stationary operand (weights) — loading
            # 64 instead of 96 columns per pass cuts LDWEIGHTS time, which is
            # the PE bottleneck; the 96-wide side streams at 2.4 GHz. ----
            bg_ps0 = psbg.tile([MLOC, NV], f32, tag="bg0")
            bg_ps1 = psbg.tile([MLOC, NV], f32, tag="bg1")
            for i in range(128):
                sl = big[:, i * NV:(i + 1) * NV]
                ps = bg_ps0 if i % 2 == 0 else bg_ps1
                nc.tensor.matmul(ps[:], sl[:, 0:MLOC], sl,
                                 start=(i < 2), stop=(i >= 126))
            bg_sb = outp.tile([MLOC, 2 * NV], f32)
            nc.vector.tensor_copy(bg_sb[:, 0:NV], bg_ps0[:])
            nc.scalar.activation(bg_sb[:, NV:2 * NV], bg_ps1[:],
                                 mybir.ActivationFunctionType.Copy)
            nc.sync.dma_start(bg_out[:], bg_sb[:])
            nc.sync.dma_start(rt_out[:], rt[:])

    nc.compile()
    return nc


def _get_nc():
    if "nc" not in _BUILT:
        _BUILT["nc"] = _build_bass()
    return _BUILT["nc"]


def _sbuf_image(a):
    """(N, L, H) row-major -> (N, 128, 512) SBUF image with
    img[n, p, g*128 + l] = a[n, l, 128*g + p]."""
    n = a.shape[0]
    return np.ascontiguousarray(
        a.reshape(n, L, G, 128).transpose(0, 3, 2, 1).reshape(n, 128, G * 128))


def _prepare_inputs(x, centroid_w):
    x64 = np.asarray(x, np.float64)
    xc = (x64 - x64.mean(axis=1, keepdims=True)).astype(np.float32)
    xt_img = _sbuf_image(xc)
    cent = np.asarray(centroid_w, np.float32).reshape(M, L, H)
    in_maps = []
    for c in range(N_CORES):
        ct_img = _sbuf_image(cent[c * MLOC:(c + 1) * MLOC])
        in_maps.append({"xt": xt_img, "ct": ct_img})
    return in_maps


def _postprocess(results):
    bgs = [r["bg"][:, 0:NV].astype(np.float64) + r["bg"][:, NV:2 * NV]
           for r in results]
    hsic = np.hstack([bg[:, MLOC:NV].T for bg in bgs])
    Q = np.concatenate([np.diagonal(bg[:, 0:MLOC]) for bg in bgs])
    R = np.hstack([r["rt"][:, 0:MLOC] for r in results]).astype(np.float64)
    vx2 = results[0]["rt"][:, MLOC:MLOC + B].sum(axis=0).astype(np.float64)
    vx = np.sqrt(vx2)
    S = R.sum(axis=0)
    sumR2 = (R * R).sum(axis=0)
    vy = np.sqrt(Q - (2.0 / L) * sumR2 + (S / L) ** 2)

    s = np.abs(hsic) / (vx[:, None] * vy[None, :])
    mat = (-np.log(s + EPS)).astype(np.float32)
    idx = np.argmax(mat, axis=1).astype(np.int32)

    hsic2 = hsic[:, idx]
    vs = vy[idx]
    loss = np.float32(
        -np.log(np.mean(np.abs(hsic2) / (vx[:, None] * vs[None, :])) + EPS))
    return loss, mat, idx


def run_spmd(in_maps, trace=False):
    from concourse.bass_utils import run_bass_kernel_spmd
    return run_bass_kernel_spmd(_get_nc(), in_maps,
                                core_ids=list(range(N_CORES)), trace=trace)


def kernel(x, centroid_w):
    in_maps = _prepare_inputs(x, centroid_w)
    res = run_spmd(in_maps)
    return _postprocess(res.results)


# revision 17
# speedup vs baseline: 1.0143x; 1.0143x over previous
"""CKA vq_codebook kernel for 8 Trainium2 NeuronCores.

Math (verified against the reference in fp64):
  Gx[b]  = x[b] @ x[b]^T, Cxc = double-center(Gx) = (P x[b]) (P x[b])^T
  Gy[m]  = cent[m] @ cent[m]^T           (uncentered)
  hsic[b,m] = <Cxc[b], center(Gy[m])> = <Cxc[b], Gy[m]>   (centering is a
              self-adjoint projection and Cxc is already centered)
  vx[b] = ||Cxc[b]||_F
  vy[m] = ||center(Gy[m])||_F = sqrt(Q - (2/L)*sum_i R_i^2 + (S/L)^2)
          with Q = ||Gy||_F^2, R = row sums of Gy, S = total sum
  The second CKA pass is a pure gather: Cs = Cy[idx]  =>  hsic2 = hsic[:, idx],
  vs = vy[idx].

Sharding: codebook M=512 split 64 per core (expert-style); every core
computes all B=32 input grams (cheaper than an AllGather on this fabric —
the 2MB collective measured ~90us end-to-end).  Each core computes a 96x64
block of the Gram of the flattened 128x128 matrices
[Gy_0..Gy_63 | Cxc_0..Cxc_31] against the 64 Gy ones: rows 64..96 give hsic,
the leading diagonal gives ||Gy||^2.  Host does the O(B*M) normalization /
log / argmax tail and the gathered second pass.

Layout tricks: the PE contracts over partitions only, so the gram G = A A^T
needs A^T (h-major) tiles in SBUF — the host pre-transposes x and the
centroids into the exact SBUF images so the device never transposes anything.
The flattened-tile matrix ("big") is stored i-major (contraction chunk major)
so the 128 accumulating Gram matmuls read contiguous SBUF columns.
"""

import os

import numpy as np

# Recover automatically if a previous process left a NeuronCore wedged; takes
# effect at NRT init (which happens after import, on first device use).
os.environ.setdefault("NEURON_RT_RESET_CORES", "1")

B, L, H, M = 32, 128, 512, 512
N_CORES = 8
MLOC = M // N_CORES          # 64 centroids per core
G = H // 128                 # 4 contraction chunks of 128
NV = B + MLOC                # 96 vectors in the per-core big Gram
EPS = 1e-8

_BUILT = {}


def _build_bass():
    """Build the single-core Bass/Tile program (same program on all 8 cores)."""
    from concourse import bacc, mybir
    from concourse.tile import TileContext

    f32 = mybir.dt.float32
    nc = bacc.Bacc("TRN2", target_bir_lowering=False, debug=False,
                   num_devices=N_CORES)

    xt = nc.dram_tensor("xt", [B, 128, G * 128], f32, kind="ExternalInput")
    ct = nc.dram_tensor("ct", [MLOC, 128, G * 128], f32, kind="ExternalInput")
    bg_out = nc.dram_tensor("bg", [MLOC, NV], f32, kind="ExternalOutput")
    rt_out = nc.dram_tensor("rt", [128, MLOC + B], f32, kind="ExternalOutput")

    with TileContext(nc) as tc:
        with (
            tc.tile_pool(name="big", bufs=1) as bigp,
            tc.tile_pool(name="stage", bufs=8) as stage,
            tc.tile_pool(name="cxs", bufs=4) as cxs,
            tc.tile_pool(name="psg", bufs=4, space="PSUM") as psg,
            tc.tile_pool(name="psbg", bufs=1, space="PSUM") as psbg,
            tc.tile_pool(name="outp", bufs=1) as outp,
        ):
            # i-major: flattened-tile vector v's chunk i lives at free column
            # i*NV + v.  Vectors 0..64 = local Gy, 64..96 = global Cxc.
            big = bigp.tile([128, 128 * NV], f32)
            big3 = big[:].rearrange("p (i v) -> p i v", v=NV)
            rt = outp.tile([128, MLOC + B], f32)

            def gram(src_dram):
                st = stage.tile([128, G * 128], f32, tag="stage")
                nc.gpsimd.dma_start(st[:], src_dram)
                ps = psg.tile([128, 128], f32, tag="gram")
                for g in range(G):
                    sl = st[:, 128 * g:128 * (g + 1)]
                    nc.tensor.matmul(ps[:], sl, sl,
                                     start=(g == 0), stop=(g == G - 1))
                return ps

            # ---- Cxc grams (all B, replicated) + vx^2 stats ----
            for b in range(B):
                ps = gram(xt[b])
                v = MLOC + b
                if b % 2 == 0:
                    nc.scalar.activation(big3[:, :, v], ps[:],
                                         mybir.ActivationFunctionType.Copy)
                else:
                    nc.vector.tensor_copy(big3[:, :, v], ps[:])
                scr = cxs.tile([128, 128], f32, tag="scr")
                nc.scalar.activation(scr[:], ps[:],
                                     mybir.ActivationFunctionType.Square)
                nc.vector.tensor_reduce(
                    rt[:, MLOC + b:MLOC + b + 1], scr[:],
                    axis=mybir.AxisListType.X, op=mybir.AluOpType.add)

            # ---- local Gy grams + row sums ----
            for m in range(MLOC):
                ps = gram(ct[m])
                if m % 2 == 0:
                    nc.scalar.activation(big3[:, :, m], ps[:],
                                         mybir.ActivationFunctionType.Copy)
                else:
                    nc.vector.tensor_copy(big3[:, :, m], ps[:])
                nc.vector.tensor_reduce(
                    rt[:, m:m + 1], ps[:],
                    axis=mybir.AxisListType.X, op=mybir.AluOpType.add)

            # ---- big Gram: [64 Gy] x [all 96] over 128 contraction chunks.
            # The 64 Gy vectors are the stationary operand (weights) — loading
            # 64 instead of 96 columns per pass cuts LDWEIGHTS time, which is
            # the PE bottleneck; the 96-wide side streams at 2.4 GHz. ----
            bg_ps = psbg.tile([MLOC, NV], f32)
            for i in range(128):
                sl = big[:, i * NV:(i + 1) * NV]
                nc.tensor.matmul(bg_ps[:], sl[:, 0:MLOC], sl,
                                 start=(i == 0), stop=(i == 127))
            bg_sb = outp.tile([MLOC, NV], f32)
            nc.vector.tensor_copy(bg_sb[:], bg_ps[:])
            nc.sync.dma_start(bg_out[:], bg_sb[:])
            nc.sync.dma_start(rt_out[:], rt[:])

    nc.compile()
    return nc


def _get_nc():
    if "nc" not in _BUILT:
        _BUILT["nc"] = _build_bass()
    return _BUILT["nc"]


def _sbuf_image(a):
    """(N, L, H) row-major -> (N, 128, 512) SBUF image with
    img[n, p, g*128 + l] = a[n, l, 128*g + p]."""
    n = a.shape[0]
    return np.ascontiguousarray(
        a.reshape(n, L, G, 128).transpose(0, 3, 2, 1).reshape(n, 128, G * 128))


def _prepare_inputs(x, centroid_w):
    x64 = np.asarray(x, np.float64)
    xc = (x64 - x64.mean(axis=1, keepdims=True)).astype(np.float32)
    xt_img = _sbuf_image(xc)
    cent = np.asarray(centroid_w, np.float32).reshape(M, L, H)
    in_maps = []
    for c in range(N_CORES):
        ct_img = _sbuf_image(cent[c * MLOC:(c + 1) * MLOC])
        in_maps.append({"xt": xt_img, "ct": ct_img})
    return in_maps


def _postprocess(results):
    hsic = np.hstack([r["bg"][:, MLOC:NV].T for r in results]).astype(np.float64)
    Q = np.concatenate(
        [np.diagonal(r["bg"][:, 0:MLOC]) for r in results]).astype(np.float64)
    R = np.hstack([r["rt"][:, 0:MLOC] for r in results]).astype(np.float64)
    vx2 = results[0]["rt"][:, MLOC:MLOC + B].sum(axis=0).astype(np.float64)
    vx = np.sqrt(vx2)
    S = R.sum(axis=0)
    sumR2 = (R * R).sum(axis=0)
    vy = np.sqrt(Q - (2.0 / L) * sumR2 + (S / L) ** 2)

    s = np.abs(hsic) / (vx[:, None] * vy[None, :])
    mat = (-np.log(s + EPS)).astype(np.float32)
    idx = np.argmax(mat, axis=1).astype(np.int32)

    hsic2 = hsic[:, idx]
    vs = vy[idx]
    loss = np.float32(
        -np.log(np.mean(np.abs(hsic2) / (vx[:, None] * vs[None, :])) + EPS))
    return loss, mat, idx


def run_spmd(in_maps, trace=False):
    from concourse.bass_utils import run_bass_kernel_spmd
    return run_bass_kernel_spmd(_get_nc(), in_maps,
                                core_ids=list(range(N_CORES)), trace=trace)


def kernel(x, centroid_w):
    in_maps = _prepare_inputs(x, centroid_w)
    res = run_spmd(in_maps)
    return _postprocess(res.results)
